# revision 1
# baseline (speedup 1.0000x reference)
"""Trainium2 Bass kernel for nn_HarModel (6-layer stacked LSTM + time-axis
LayerNorms + dense softmax heads).

Contract: kernel(**inputs) takes FULL unsharded numpy inputs (as produced by
setup_inputs()) and returns the FULL [1024, 5] float32 output.

Strategy: data-parallel over batch across 8 NeuronCores (B=128 per core),
LSTM recurrence unrolled over T=256 steps per layer, feature-major layout
(gate/hidden units on partitions, batch on the free dim) so the recurrent
matmul h @ U needs no transposes. Matmuls run in bf16 (fp32 PSUM accum).

Key tricks:
  - gate columns permuted to [i, f, o, g] so one sigmoid op covers i,f,o
    and one relu op covers g.
  - biases folded into the matmuls as an extra K row against a ones-row
    (except layer 3 where K would exceed 128 -> per-gate ACT bias there).
  - c_t >= 0 by induction (f,i in (0,1), relu(g) >= 0, c_0 = 0), so
    relu(c) == c and h = o * c directly.
  - cell update fused to 3 DVE ops: [u;p] = [i';f'] * [g';c], c' = u + p,
    h = o' * c'.
"""

import numpy as np
import ml_dtypes
from contextlib import ExitStack

import concourse.bass as bass
import concourse.bacc as bacc
import concourse.tile as tile
import concourse.mybir as mybir
import concourse.bass_utils as bass_utils

AF = mybir.ActivationFunctionType
ALU = mybir.AluOpType
BF16 = mybir.dt.bfloat16
F32 = mybir.dt.float32
nbf16 = ml_dtypes.bfloat16

NCORES = 8
B = 128          # batch per core
EPS = 1e-3

# (din, H) per LSTM layer
DIMS = [(32, 64), (64, 128), (128, 128), (128, 64), (64, 64), (64, 64)]
# gate block permutation: keras order [i, f, g, o] -> ours [i, f, o, g]
GPERM = [0, 1, 3, 2]

_CACHE = {}


def _permute_gates(w, H):
    """Reorder the 4H gate columns from [i,f,g,o] to [i,f,o,g]."""
    blocks = [w[..., g * H:(g + 1) * H] for g in GPERM]
    return np.concatenate(blocks, axis=-1)


def _build(T, ln_trivial):
    """Build + compile the Bass program. ln_trivial: 3 bools (gamma==1 and
    beta==0 for each of the 3 LayerNorms)."""
    nc = bacc.Bacc("TRN2", target_bir_lowering=False, debug=False,
                   num_devices=NCORES)

    # ---------------- DRAM tensors ----------------
    xp_d = nc.dram_tensor("xp", [33, T, B], BF16, kind="ExternalInput").ap()
    w_d, u_d = [], []
    # layer configs: (din, H, M, w_bias_row, u_bias_row, per_gate_act)
    LCFG = [
        (32, 64, 64, True, False, False),
        (64, 128, 128, True, False, False),
        (128, 128, 128, False, False, True),
        (128, 64, 64, False, True, False),
        (64, 64, 64, True, False, False),
        (64, 64, 64, True, False, False),
    ]
    for l, (din, H, M, wbr, ubr, pga) in enumerate(LCFG):
        kw = din + (1 if wbr else 0)
        ku = H + (1 if ubr else 0)
        w_d.append(nc.dram_tensor(f"w{l}", [kw, 4 * H], BF16,
                                  kind="ExternalInput").ap())
        u_d.append(nc.dram_tensor(f"u{l}", [ku, 4 * H], BF16,
                                  kind="ExternalInput").ap())
    b2t_d = nc.dram_tensor("b2t", [128, 4], F32, kind="ExternalInput").ap()
    ln_g_d, ln_b_d = {}, {}
    for i in range(3):
        if not ln_trivial[i]:
            ln_g_d[i] = nc.dram_tensor(f"lng{i}", [128, T], BF16,
                                       kind="ExternalInput").ap()
            ln_b_d[i] = nc.dram_tensor(f"lnb{i}", [128, T], BF16,
                                       kind="ExternalInput").ap()
    dw1p_d = nc.dram_tensor("dw1p", [65, 64], BF16, kind="ExternalInput").ap()
    dw2p_d = nc.dram_tensor("dw2p", [65, 32], BF16, kind="ExternalInput").ap()
    owp_d = nc.dram_tensor("owp", [33, 5], BF16, kind="ExternalInput").ap()
    eye5_d = nc.dram_tensor("eye5", [5, 5], F32, kind="ExternalInput").ap()
    out_d = nc.dram_tensor("out", [B, 5], F32, kind="ExternalOutput").ap()

    with tile.TileContext(nc) as tc:
        with ExitStack() as ctx:
            big = ctx.enter_context(tc.tile_pool(name="big", bufs=1))
            wp = ctx.enter_context(tc.tile_pool(name="wp", bufs=1))
            zp = ctx.enter_context(tc.tile_pool(
                name="zp", bufs=4, space=bass.MemorySpace.PSUM))
            sp = ctx.enter_context(tc.tile_pool(name="sp", bufs=4))
            gp = ctx.enter_context(tc.tile_pool(name="gp", bufs=1))
            st = ctx.enter_context(tc.tile_pool(name="st", bufs=1))

            buf_a = big.tile([128, T, B], BF16, tag="bufa")
            buf_b = big.tile([128, T, B], BF16, tag="bufb")

            wt, ut = [], []
            for l, (din, H, M, wbr, ubr, pga) in enumerate(LCFG):
                w = wp.tile(list(w_d[l].shape), BF16, tag=f"w{l}")
                u = wp.tile(list(u_d[l].shape), BF16, tag=f"u{l}")
                nc.sync.dma_start(w[:], w_d[l])
                nc.sync.dma_start(u[:], u_d[l])
                wt.append(w)
                ut.append(u)
            b2t = wp.tile([128, 4], F32, tag="b2t")
            nc.sync.dma_start(b2t[:], b2t_d)
            ln_g, ln_b = {}, {}
            for i in range(3):
                if not ln_trivial[i]:
                    ln_g[i] = wp.tile([128, T], BF16, tag=f"lng{i}",
                                      name=f"lng{i}")
                    ln_b[i] = wp.tile([128, T], BF16, tag=f"lnb{i}",
                                      name=f"lnb{i}")
                    nc.sync.dma_start(ln_g[i][:], ln_g_d[i])
                    nc.sync.dma_start(ln_b[i][:], ln_b_d[i])
            dw1p = wp.tile([65, 64], BF16, tag="dw1p")
            dw2p = wp.tile([65, 32], BF16, tag="dw2p")
            owp = wp.tile([33, 5], BF16, tag="owp")
            eye5 = wp.tile([5, 5], F32, tag="eye5")
            nc.sync.dma_start(dw1p[:], dw1p_d)
            nc.sync.dma_start(dw2p[:], dw2p_d)
            nc.sync.dma_start(owp[:], owp_d)
            nc.sync.dma_start(eye5[:], eye5_d)

            nc.sync.dma_start(buf_a[0:33, :, :], xp_d)

            h_init0 = wp.tile([128, B], BF16, tag="hinit0")
            nc.vector.memset(h_init0[:], 0.0)
            h_init4 = wp.tile([65, B], BF16, tag="hinit4")
            nc.vector.memset(h_init4[:], 0.0)
            nc.vector.memset(h_init4[64:65, :], 1.0)
            epsb = wp.tile([128, 1], F32, tag="epsb")
            nc.vector.memset(epsb[:], EPS)

            def lstm_layer(l, inbuf, in_rows, outbuf):
                din, H, M, wbr, ubr, pga = LCFG[l]
                nch = (4 * H) // M          # chunks (always 4)
                kw = din + (1 if wbr else 0)
                ku = H + (1 if ubr else 0)
                assert in_rows == kw
                G = 2           # independent sub-batch chains for pipelining
                Bg = B // G
                gcl = []
                for g in range(G):
                    gt = gp.tile([128, 2 * (B // G)], BF16, tag=f"gc{g}",
                                 name=f"gc{g}")
                    nc.vector.memset(gt[0:M, :], 0.0)  # c_0 = 0
                    gcl.append(gt)
                # per-chain ping-pong h tiles: the recurrence reads/writes
                # these small tiles only; h is copied into the big sequence
                # buffer off the critical path (gpsimd).
                hpp = []
                for g in range(G):
                    row = []
                    for p in range(2):
                        ht = gp.tile([128, Bg], BF16, tag=f"hpp{g}{p}",
                                     name=f"hpp{g}{p}")
                        if ubr:
                            nc.vector.memset(ht[64:65, :], 1.0)
                        row.append(ht)
                    hpp.append(row)
                # chain-blocked layouts: chain g owns contiguous column
                # blocks in z/s/up/gc so dep ranges are disjoint and the G
                # recurrences pipeline freely across PE/ACT/DVE.
                for t in range(T):
                    for g in range(G):
                        lo, hi = g * Bg, (g + 1) * Bg
                        zc, sc = g * 4 * Bg, g * 3 * Bg
                        uc = g * 2 * Bg
                        z = zp.tile([128, 4 * B], F32, tag="z", name="z")
                        for c in range(nch):
                            nc.tensor.matmul(
                                z[0:M, zc + c * Bg:zc + (c + 1) * Bg],
                                wt[l][:, c * M:(c + 1) * M],
                                inbuf[0:kw, t, lo:hi],
                                start=True, stop=False)
                            h0 = h_init4 if ubr else h_init0
                            rhs = (h0[0:ku, lo:hi] if t == 0
                                   else hpp[g][(t - 1) % 2][0:ku, :])
                            nc.tensor.matmul(
                                z[0:M, zc + c * Bg:zc + (c + 1) * Bg],
                                ut[l][:, c * M:(c + 1) * M],
                                rhs,
                                start=False, stop=True)
                        s = sp.tile([128, 3 * B], BF16, tag="s", name="s")
                        if pga:
                            for c in range(3):
                                nc.scalar.activation(
                                    s[0:M, sc + c * Bg:sc + (c + 1) * Bg],
                                    z[0:M, zc + c * Bg:zc + (c + 1) * Bg],
                                    AF.Sigmoid, bias=b2t[0:M, c:c + 1])
                            nc.scalar.activation(
                                gcl[g][0:M, 0:Bg],
                                z[0:M, zc + 3 * Bg:zc + 4 * Bg],
                                AF.Relu, bias=b2t[0:M, 3:4])
                        else:
                            nc.scalar.activation(
                                s[0:M, sc:sc + 3 * Bg],
                                z[0:M, zc:zc + 3 * Bg], AF.Sigmoid)
                            nc.scalar.activation(
                                gcl[g][0:M, 0:Bg],
                                z[0:M, zc + 3 * Bg:zc + 4 * Bg], AF.Relu)
                        up = sp.tile([128, 2 * B], BF16, tag="up", name="up")
                        # [u; p] = [i'; f'] * [g'; c]
                        nc.vector.tensor_mul(
                            up[0:M, uc:uc + 2 * Bg],
                            s[0:M, sc:sc + 2 * Bg],
                            gcl[g][0:M, 0:2 * Bg])
                        # c' = u + p
                        nc.vector.tensor_add(
                            gcl[g][0:M, Bg:2 * Bg],
                            up[0:M, uc:uc + Bg],
                            up[0:M, uc + Bg:uc + 2 * Bg])
                        # h = o' * c'   (relu(c)==c since c>=0)
                        nc.vector.tensor_mul(
                            hpp[g][t % 2][0:H, :],
                            s[0:M, sc + 2 * Bg:sc + 3 * Bg],
                            gcl[g][0:M, Bg:2 * Bg])
                        nc.gpsimd.tensor_copy(
                            outbuf[0:H, t, lo:hi], hpp[g][t % 2][0:H, :])

            def emit_ln(idx, Y, H, OUT, scratch):
                """LN over time axis. Y/OUT/scratch: [H,T,B] bf16 AP slices
                (scratch may alias OUT)."""
                sm = st.tile([128, B], F32, tag="sm")
                sq = st.tile([128, B], F32, tag="sq")
                yv = Y.rearrange("p t b -> p b t")
                nc.vector.tensor_reduce(sm[0:H, :], yv, mybir.AxisListType.X,
                                        ALU.add)
                nc.vector.tensor_mul(scratch, Y, Y)
                sv = scratch.rearrange("p t b -> p b t")
                nc.vector.tensor_reduce(sq[0:H, :], sv, mybir.AxisListType.X,
                                        ALU.add)
                mu = st.tile([128, B], F32, tag="mu")
                var = st.tile([128, B], F32, tag="var")
                nc.vector.tensor_scalar_mul(mu[0:H, :], sm[0:H, :], 1.0 / T)
                nc.vector.tensor_scalar_mul(sq[0:H, :], sq[0:H, :], 1.0 / T)
                nc.vector.tensor_mul(var[0:H, :], mu[0:H, :], mu[0:H, :])
                nc.vector.tensor_sub(var[0:H, :], sq[0:H, :], var[0:H, :])
                sd = st.tile([128, B], F32, tag="sd")
                nc.scalar.activation(sd[0:H, :], var[0:H, :], AF.Sqrt,
                                     bias=epsb[0:H, :])
                rr = st.tile([128, B], F32, tag="rr")
                nc.vector.reciprocal(rr[0:H, :], sd[0:H, :])
                mr = st.tile([128, B], F32, tag="mr")
                nc.vector.tensor_mul(mr[0:H, :], mu[0:H, :], rr[0:H, :])
                r16 = st.tile([128, B], BF16, tag="r16")
                m16 = st.tile([128, B], BF16, tag="m16")
                nc.vector.tensor_copy(r16[0:H, :], rr[0:H, :])
                nc.vector.tensor_copy(m16[0:H, :], mr[0:H, :])
                rb = r16[0:H, :].unsqueeze(1).broadcast_to((H, T, B))
                mb = m16[0:H, :].unsqueeze(1).broadcast_to((H, T, B))
                nc.vector.tensor_mul(OUT, Y, rb)
                nc.vector.tensor_sub(OUT, OUT, mb)
                if idx in ln_g:
                    gb = ln_g[idx][0:H, :].unsqueeze(2).broadcast_to((H, T, B))
                    bb = ln_b[idx][0:H, :].unsqueeze(2).broadcast_to((H, T, B))
                    nc.vector.tensor_mul(OUT, OUT, gb)
                    nc.vector.tensor_add(OUT, OUT, bb)

            # ---------------- the network ----------------
            lstm_layer(0, buf_a, 33, buf_b)                       # h1 -> B
            emit_ln(0, buf_b[0:64, :, :], 64,
                    buf_a[0:64, :, :], buf_a[0:64, :, :])         # ln1 -> A
            nc.vector.memset(buf_a[64:65, :, :], 1.0)
            lstm_layer(1, buf_a, 65, buf_b)                       # h2 -> B
            lstm_layer(2, buf_b, 128, buf_a)                      # h3 -> A
            emit_ln(1, buf_a[0:128, :, :], 128,
                    buf_b[0:128, :, :], buf_b[0:128, :, :])       # ln2 -> B
            nc.vector.memset(buf_a[64:65, :, :], 1.0)
            lstm_layer(3, buf_b, 128, buf_a)                      # h4 -> A
            lstm_layer(4, buf_a, 65, buf_b)                       # h5 -> B
            emit_ln(2, buf_b[0:64, :, :], 64,
                    buf_a[0:64, :, :], buf_a[0:64, :, :])         # ln3 -> A
            nc.vector.memset(buf_a[64:65, :, :], 1.0)
            lstm_layer(5, buf_a, 65, buf_b)                       # h6 -> B

            # ---------------- dense head ----------------
            nc.vector.memset(buf_b[64:65, T - 1:T, :], 1.0)
            h6e = buf_b[0:65, T - 1, :]
            pd1 = zp.tile([128, 4 * B], F32, tag="z")
            nc.tensor.matmul(pd1[0:64, 0:B], dw1p[:, :], h6e,
                             start=True, stop=True)
            d1e = sp.tile([128, 3 * B], BF16, tag="s")
            nc.vector.memset(d1e[64:65, 0:B], 1.0)
            nc.scalar.activation(d1e[0:64, 0:B], pd1[0:64, 0:B], AF.Relu)
            pd2 = zp.tile([128, 4 * B], F32, tag="z")
            nc.tensor.matmul(pd2[0:32, 0:B], dw2p[:, :], d1e[0:65, 0:B],
                             start=True, stop=True)
            d2e = sp.tile([128, 3 * B], BF16, tag="s")
            nc.vector.memset(d2e[32:33, 0:B], 1.0)
            nc.scalar.activation(d2e[0:32, 0:B], pd2[0:32, 0:B], AF.Relu)
            plog = zp.tile([128, 4 * B], F32, tag="z")
            nc.tensor.matmul(plog[0:5, 0:B], owp[:, :], d2e[0:33, 0:B],
                             start=True, stop=True)
            lsb = st.tile([128, B], F32, tag="lsb")
            nc.scalar.copy(lsb[0:5, :], plog[0:5, 0:B])
            ptr = zp.tile([128, 4 * B], F32, tag="z")
            nc.tensor.matmul(ptr[0:B, 0:5], lsb[0:5, :], eye5[:, :],
                             start=True, stop=True, is_transpose=True)
            ex = st.tile([128, 8], F32, tag="ex")
            nc.scalar.activation(ex[:, 0:5], ptr[0:B, 0:5], AF.Exp)
            s1 = st.tile([128, 1], F32, tag="s1")
            s2 = st.tile([128, 1], F32, tag="s2")
            nc.vector.tensor_reduce(s1[:], ex[:, 0:3], mybir.AxisListType.X,
                                    ALU.add)
            nc.vector.tensor_reduce(s2[:], ex[:, 3:5], mybir.AxisListType.X,
                                    ALU.add)
            nc.vector.reciprocal(s1[:], s1[:])
            nc.vector.reciprocal(s2[:], s2[:])
            outf = st.tile([128, 8], F32, tag="outf")
            nc.vector.tensor_scalar_mul(outf[:, 0:3], ex[:, 0:3], s1[:])
            nc.vector.tensor_scalar_mul(outf[:, 3:5], ex[:, 3:5], s2[:])
            nc.sync.dma_start(out_d, outf[:, 0:5])

    nc.compile()
    return nc


def _prep_inputs(inputs, T):
    """Host-side packing: returns list of 8 per-core in_maps."""
    x = np.asarray(inputs["x"], np.float32)
    Btot = x.shape[0]
    ws, us = [], []
    LBR = [("w", None), ("w", None), (None, None), (None, "u"),
           ("w", None), ("w", None)]
    for l, (din, H) in enumerate(DIMS):
        W = _permute_gates(np.asarray(inputs[f"w{l+1}"], np.float32), H)
        U = _permute_gates(np.asarray(inputs[f"u{l+1}"], np.float32), H)
        bb = _permute_gates(np.asarray(inputs[f"b{l+1}"], np.float32)[None],
                            H)[0]
        wbr, ubr = LBR[l]
        if wbr == "w":
            W = np.vstack([W, bb[None, :]])
        if ubr == "u":
            U = np.vstack([U, bb[None, :]])
        ws.append(W.astype(nbf16))
        us.append(U.astype(nbf16))
    b3 = _permute_gates(np.asarray(inputs["b3"], np.float32)[None], 128)[0]
    b2t = b3.reshape(4, 128).T.copy().astype(np.float32)

    common = {}
    for l in range(6):
        common[f"w{l}"] = ws[l]
        common[f"u{l}"] = us[l]
    common["b2t"] = b2t
    for i in range(3):
        g = np.asarray(inputs[f"g{i+1}"], np.float32)
        be = np.asarray(inputs[f"be{i+1}"], np.float32)
        if not (np.all(g == 1.0) and np.all(be == 0.0)):
            common[f"lng{i}"] = np.tile(g[None, :], (128, 1)).astype(nbf16)
            common[f"lnb{i}"] = np.tile(be[None, :], (128, 1)).astype(nbf16)
    dw1 = np.asarray(inputs["dw1"], np.float32)
    db1 = np.asarray(inputs["db1"], np.float32)
    dw2 = np.asarray(inputs["dw2"], np.float32)
    db2 = np.asarray(inputs["db2"], np.float32)
    ow1 = np.asarray(inputs["ow1"], np.float32)
    ob1 = np.asarray(inputs["ob1"], np.float32)
    ow2 = np.asarray(inputs["ow2"], np.float32)
    ob2 = np.asarray(inputs["ob2"], np.float32)
    common["dw1p"] = np.vstack([dw1, db1[None, :]]).astype(nbf16)
    common["dw2p"] = np.vstack([dw2, db2[None, :]]).astype(nbf16)
    ow = np.hstack([ow1, ow2])
    ob = np.concatenate([ob1, ob2])
    common["owp"] = np.vstack([ow, ob[None, :]]).astype(nbf16)
    common["eye5"] = np.eye(5, dtype=np.float32)

    in_maps = []
    bs = Btot // NCORES
    for k in range(NCORES):
        xs = x[k * bs:(k + 1) * bs]                      # [B, T, F]
        xt = np.ascontiguousarray(xs.transpose(2, 1, 0)) # [F, T, B]
        xp = np.concatenate(
            [xt, np.ones((1, T, bs), np.float32)], axis=0).astype(nbf16)
        m = dict(common)
        m["xp"] = xp
        in_maps.append(m)
    return in_maps


def _ln_trivial_key(inputs):
    out = []
    for i in range(3):
        g = np.asarray(inputs[f"g{i+1}"], np.float32)
        be = np.asarray(inputs[f"be{i+1}"], np.float32)
        out.append(bool(np.all(g == 1.0) and np.all(be == 0.0)))
    return tuple(out)


def get_program(T, ln_trivial):
    key = (T, ln_trivial)
    if key not in _CACHE:
        _CACHE[key] = _build(T, ln_trivial)
    return _CACHE[key]


def kernel(**inputs):
    x = np.asarray(inputs["x"])
    T = x.shape[1]
    ln_trivial = _ln_trivial_key(inputs)
    nc = get_program(T, ln_trivial)
    in_maps = _prep_inputs(inputs, T)
    res = bass_utils.run_bass_kernel_spmd(
        nc, in_maps, core_ids=list(range(NCORES)))
    out = np.concatenate([r["out"] for r in res.results], axis=0)
    return out.astype(np.float32)

